# revision 30
# baseline (speedup 1.0000x reference)
"""Trainium2 Bass kernel for nn_Bottleneck_11416023073044 (RFAConv bottleneck).

Sharding: pure data parallelism - 1 batch sample per NeuronCore (8 cores).

Per-core pipeline (channel-major layouts, [partition, spatial] tiles).
Partition interleave is cl-major: p = cl*9 + n for channel-in-group cl and
patch index n.  Groups of GC=14 channels (last group has 2).

  cv1:    ph = W1' @ x  (W1 pre-scaled by BN a1), PSUM
          sg = Sigmoid(ph + c1)       ACT, bias=c1
          hp = (ph + c1) * sg         DVE scalar_tensor_tensor -> bf16 frame
  hp9d:   3 DMAs per chunk write the 9 shifted flat windows (grouped by
          row-shift a, the 3 col-shifts b are one AP dim) to DRAM
  strips: per block (20 rows) per group g: one gather DMA builds
          st_g[(cl,n), r, x] = hp[c, (ys+r)*82 + x + d_n]
  z:      pz = zb_g^T @ st            (bf16, PSUM fp32)
  e:      e9 = exp(pz + cg9)          ACT -> bf16 (2-chunk buffer)
  D:      pd = sum over groups of dones_g^T @ e9_g  (one PSUM accumulation
          over all 10 groups; output partition = channel)
  rcp:    rcp2[ch] = 1/pd             DVE reciprocal -> bf16
  q1:     e9q1_g *= st_g              DVE in-place (patches * e)
  rep:    rep_g = SBUF->SBUF DMA broadcast of rcp2 rows (0-stride over n)
  q2:     q2_g = e9q1_g * rep_g       DVE
  out:    po[h] += wc9_g^T @ q2_g     bf16 matmuls over groups
  final:  t = relu(a2*po + c2) ACT;  og = x + t (Pool);  DMA out
"""
import numpy as np
import ml_dtypes

EPS = 1e-5
B, C1, C2, H, W = 8, 256, 256, 80, 80
C_ = C2 // 2          # 128
NG = 10               # channel groups
GC = 14               # channels per group (last group has 2)
HP = H + 2            # 82
S = H * W             # 6400
ROWS_BK = 5           # frame rows per PSUM bank (400 cols)
CSB = ROWS_BK * W     # 400
CH_ROWS = 2 * ROWS_BK # rows per compute chunk (2 banks)
CS = CH_ROWS * W      # 800 columns per chunk
NCH = H // CH_ROWS    # 8 chunks
BLK_ROWS = 20         # rows per block (2 chunks)
NBLK = H // BLK_ROWS  # 4 blocks
BW = BLK_ROWS * HP    # 1640: strip window per block
FW = 80 * HP          # 6560: flat window length per shifted copy
HPF = HP * HP + 2     # 6726 flat frame length (tail padded)


def _grp(g):
    c0 = g * GC
    ncg = min(GC, C_ - c0)
    return c0, ncg, 9 * ncg


def _fold_constants(W1, g1, b1, m1, v1, Wg, bg, gg, bgw, mg, vg, Wc, bc, g2, b2,
                    m2, v2):
    """Fold BN affines and build the cl-major-layout stationaries.

    Partition index p = cl*9 + n for channel-in-group cl, patch index n.
    """
    f32 = np.float32
    bf16 = ml_dtypes.bfloat16
    cst = {}
    a1 = (g1 / np.sqrt(v1 + EPS)).astype(f32)
    c1 = (b1 - m1 * a1).astype(f32)
    cst['c1sig'] = c1.reshape(C_, 1)

    w1t = np.zeros((C_, 2, C_), f32)                          # [c_in, t, o]
    for t in range(2):
        w1t[:, t, :] = (a1[:, None] * W1[:, t * C_:(t + 1) * C_]).T
    cst['w1t'] = w1t.astype(bf16)

    ag = gg / np.sqrt(vg + EPS)                               # [128, 9]
    A = (ag[:, :, None] * Wg).astype(f32)                     # [c, n, i]
    cg = (ag * (bg - mg) + bgw).astype(f32)                   # [128, 9]

    zb = np.zeros((126, NG, 126), f32)
    cg9 = np.zeros((126, NG), f32)
    wc9 = np.zeros((126, NG, C2), f32)
    dones = np.zeros((126, NG, C_), f32)
    for g in range(NG):
        c0, ncg, P = _grp(g)
        for cl in range(ncg):
            c = c0 + cl
            for n in range(9):
                m = cl * 9 + n
                cg9[m, g] = cg[c, n]
                wc9[m, g, :] = Wc[:, c, n]
                dones[m, g, c] = 1.0
                for i in range(9):
                    zb[cl * 9 + i, g, m] = A[c, n, i]
    cst['zb'] = zb.astype(bf16)
    cst['cg9'] = cg9
    cst['wc9'] = wc9.astype(bf16)
    cst['dones'] = dones.astype(bf16)

    a2 = (g2 / np.sqrt(v2 + EPS)).astype(f32)
    c2 = (b2 + a2 * (bc - m2)).astype(f32)
    a2c2 = np.zeros((C_, 2, 2), f32)
    for h in range(2):
        a2c2[:, h, 0] = a2[h * C_:(h + 1) * C_]
        a2c2[:, h, 1] = c2[h * C_:(h + 1) * C_]
    cst['a2c2'] = a2c2
    return cst


_PROGRAM = None


def _build_program():
    import concourse.bass as bass
    import concourse.tile as tile
    from concourse import mybir

    dt = mybir.dt
    AF = mybir.ActivationFunctionType
    ALU = mybir.AluOpType

    nc = bass.Bass("TRN2", target_bir_lowering=False, debug=False)

    xs_d = nc.dram_tensor("xs", [C_, 2, S], dt.bfloat16, kind="ExternalInput")
    w1t_d = nc.dram_tensor("w1t", [C_, 2, C_], dt.bfloat16, kind="ExternalInput")
    c1_d = nc.dram_tensor("c1sig", [C_, 1], dt.float32, kind="ExternalInput")
    zb_d = nc.dram_tensor("zb", [126, NG, 126], dt.bfloat16, kind="ExternalInput")
    dones_d = nc.dram_tensor("dones", [126, NG, C_], dt.bfloat16, kind="ExternalInput")
    cg9_d = nc.dram_tensor("cg9", [126, NG], dt.float32, kind="ExternalInput")
    wc9_d = nc.dram_tensor("wc9", [126, NG, C2], dt.bfloat16, kind="ExternalInput")
    a2c2_d = nc.dram_tensor("a2c2", [C_, 2, 2], dt.float32, kind="ExternalInput")
    out_d = nc.dram_tensor("out", [C_, 2, S], dt.bfloat16, kind="ExternalOutput")
    hp9d = nc.dram_tensor("hp9d", [9, C_, FW], dt.bfloat16)

    with tile.TileContext(nc) as tc:
        with tc.tile_pool(name="singles", bufs=1) as singles, \
             tc.tile_pool(name="strips", bufs=1) as strips, \
             tc.tile_pool(name="eq", bufs=1) as eqpool, \
             tc.tile_pool(name="q2p", bufs=1) as q2pool, \
             tc.tile_pool(name="work", bufs=3) as work, \
             tc.tile_pool(name="repp", bufs=6) as repp, \
             tc.tile_pool(name="rcpp", bufs=2) as rcpp, \
             tc.tile_pool(name="ogp", bufs=2) as ogp, \
             tc.tile_pool(name="psz", bufs=2, space="PSUM") as psz, \
             tc.tile_pool(name="psd", bufs=1, space="PSUM") as psd, \
             tc.tile_pool(name="pso", bufs=1, space="PSUM") as pso:

            # ---- resident tiles + constant loads ----
            # SP front-loads the cv1 critical path chunk by chunk
            x2 = singles.tile([C_, 2, S], dt.bfloat16, tag="x2", name="x2")
            nc.sync.dma_start(out=x2[:, :, 0:CS], in_=xs_d[:, :, 0:CS])
            w1t = singles.tile([C_, 2, C_], dt.bfloat16, tag="w1t", name="w1t")
            nc.sync.dma_start(out=w1t[:], in_=w1t_d[:])
            c1sig = singles.tile([C_, 1], dt.float32, tag="c1sig", name="c1sig")
            nc.gpsimd.dma_start(out=c1sig[:], in_=c1_d[:])
            atl = singles.tile([1, 2], dt.float32, tag="atl", name="atl")
            nc.scalar.activation(out=atl[0:1, 0:1], in_=c1sig[0:1, 0:1],
                                 func=AF.Sigmoid)
            nc.sync.dma_start(out=x2[:, :, CS:2 * CS], in_=xs_d[:, :, CS:2 * CS])
            zb = singles.tile([126, NG, 126], dt.bfloat16, tag="zb", name="zb")
            nc.gpsimd.dma_start(out=zb[:], in_=zb_d[:])
            nc.sync.dma_start(out=x2[:, :, 2 * CS:4 * CS],
                              in_=xs_d[:, :, 2 * CS:4 * CS])
            nc.gpsimd.dma_start(out=x2[:, :, 4 * CS:6 * CS],
                                in_=xs_d[:, :, 4 * CS:6 * CS])
            nc.gpsimd.dma_start(out=x2[:, :, 6 * CS:S],
                                in_=xs_d[:, :, 6 * CS:S])
            cg9 = singles.tile([126, NG], dt.float32, tag="cg9", name="cg9")
            nc.gpsimd.dma_start(out=cg9[:], in_=cg9_d[:])
            dones = singles.tile([126, NG, C_], dt.bfloat16, tag="dones", name="dones")
            nc.gpsimd.dma_start(out=dones[:], in_=dones_d[:])
            wc9 = singles.tile([126, NG, C2], dt.bfloat16, tag="wc9", name="wc9")
            a2c2 = singles.tile([C_, 2, 2], dt.float32, tag="a2c2", name="a2c2")

            hpfl = singles.tile([C_, HPF], dt.bfloat16, tag="hp", name="hp")
            hp = hpfl[:, 0:HP * HP].rearrange("p (a b) -> p a b", a=HP)
            nc.vector.memset(hp[:, 0, :], 0.0)
            nc.vector.memset(hp[:, HP - 1, :], 0.0)
            nc.vector.memset(hp[:, 1:HP - 1, 0], 0.0)
            nc.vector.memset(hp[:, 1:HP - 1, HP - 1], 0.0)
            nc.vector.memset(hpfl[:, HP * HP:], 0.0)

            # per-group tiles (allocated up front, bufs=1 semantics)
            st = []
            eq = []
            q2 = []
            for g in range(NG):
                c0, ncg, P = _grp(g)
                st.append(strips.tile([P, BLK_ROWS, HP], dt.bfloat16,
                                      tag=f"st{g}", name=f"st{g}"))
                eq.append(eqpool.tile([P, 2, 2, CSB], dt.bfloat16,
                                      tag=f"eq{g}", name=f"eq{g}"))
                q2.append(q2pool.tile([P, 2, 2, CSB], dt.bfloat16,
                                      tag=f"q2{g}", name=f"q2{g}"))

            # ---- phase A: cv1 chunks + incremental hp9d writes ----
            def cv1_chunk(ch, pool=None, tag="pz"):
                y0 = ch * CH_ROWS
                ph = (pool or psz).tile([C_, 2, 512], dt.float32, tag=tag,
                                        name="ph")
                for k in range(2):
                    cols = slice((2 * ch + k) * CSB, (2 * ch + k + 1) * CSB)
                    for t in range(2):
                        nc.tensor.matmul(
                            out=ph[:, k, 0:CSB],
                            lhsT=w1t[:, t, :],
                            rhs=x2[:, t, cols],
                            start=(t == 0), stop=(t == 1))
                sg = work.tile([C_, 2, CSB], dt.bfloat16, tag="sg", name="sg")
                nc.scalar.activation(out=sg[:], in_=ph[:, :, 0:CSB],
                                     func=AF.Sigmoid, bias=c1sig[:, 0:1])
                nc.vector.scalar_tensor_tensor(
                    out=hp[:, 1 + y0:1 + y0 + CH_ROWS, 1:1 + W],
                    in0=ph[:, :, 0:CSB], scalar=c1sig[:, 0:1], in1=sg[:],
                    op0=ALU.add, op1=ALU.mult)
                return sg

            def hp9d_incr(ch):
                # write copy cols [lo, hi): 3 DMAs, one per row-shift a, each
                # on its own queue; the col-shifts b ride as an AP dim.
                lo = 0 if ch == 0 else 820 * ch - 84
                hi = 820 * ch + 736 if ch < NCH - 1 else FW
                ln = hi - lo
                wengs = [nc.sync, nc.gpsimd,
                         nc.sync if ch % 2 == 0 else nc.gpsimd]
                for a in range(3):
                    dst = bass.AP(tensor=hp9d[0].tensor,
                                  offset=(3 * a) * (C_ * FW) + lo,
                                  ap=[[FW, C_], [C_ * FW, 3], [1, ln]])
                    src = bass.AP(tensor=hpfl[:].tensor,
                                  offset=a * HP + lo,
                                  ap=[[HPF, C_], [1, 3], [1, ln]])
                    wengs[a].dma_start(out=dst, in_=src)

            def strip_gather(b, g, eng, r0=0, r1=BLK_ROWS):
                ys = b * BLK_ROWS + r0
                c0, ncg, P = _grp(g)
                ln = (r1 - r0) * HP
                srcap = bass.AP(
                    tensor=hp9d[0].tensor, offset=c0 * FW + ys * HP,
                    ap=[[FW, ncg], [C_ * FW, 9], [1, ln]])
                eng.dma_start(
                    out=st[g][:, r0:r1, :].rearrange("p a b -> p (a b)"),
                    in_=srcap)

            cvpool = {2: (pso, "po"), 3: (psd, "pd"), 6: (pso, "po"),
                      7: (psd, "pd")}
            sg_last = None
            for ch in range(NCH):
                pl, tg = cvpool.get(ch, (None, "pz"))
                sg_last = cv1_chunk(ch, pool=pl, tag=tg)
                hp9d_incr(ch)
                if ch == 1:
                    for g in range(NG):
                        strip_gather(0, g, [nc.scalar, nc.sync, nc.gpsimd][g % 3],
                                     0, CH_ROWS)
                if ch == 2:
                    for g in range(NG):
                        strip_gather(0, g, [nc.sync, nc.gpsimd, nc.scalar][g % 3],
                                     CH_ROWS, BLK_ROWS)
                    nc.gpsimd.dma_start(out=wc9[:], in_=wc9_d[:])
                    nc.gpsimd.dma_start(out=a2c2[:], in_=a2c2_d[:])
            nc.scalar.activation(out=atl[0:1, 1:2], in_=sg_last[0:1, 0:1, 0:1],
                                 func=AF.Exp)

            og_cur = [None]
            po_cur = [None]

            def out_mms(ch, h, j0, j1, pool=None, tag="po"):
                # a 5-matmul slice of the skewed out conv's po accumulation
                # for chunk ch, output half h (20 matmuls = 10 groups x 2
                # banks, emitted in (g, k) order)
                b, cb = divmod(ch, 2)
                if j0 == 0 and h == 0:
                    og_cur[0] = ogp.tile([C_, 2, CS], dt.bfloat16, tag="og",
                                         name="og")
                if j0 == 0:
                    po_cur[0] = (pool or pso).tile([C_, 2, 512], dt.float32,
                                                   tag=tag, name="po")
                po = po_cur[0]
                for j in range(j0, j1):
                    g, k = divmod(j, 2)
                    c0, ncg, P = _grp(g)
                    nc.tensor.matmul(
                        out=po[:, k, 0:CSB],
                        lhsT=wc9[0:P, g, h * C_:(h + 1) * C_],
                        rhs=q2[g][:, cb, k, :],
                        start=(g == 0), stop=(g == NG - 1),
                        skip_group_check=True)

            def out_epi(ch, h, adde=None):
                b, cb = divmod(ch, 2)
                og = og_cur[0]
                po = po_cur[0]
                t = work.tile([C_, 2, CSB], dt.bfloat16, tag=f"t{h}",
                              name=f"t{h}")
                nc.scalar.activation(out=t[:], in_=po[:, :, 0:CSB],
                                     func=AF.Relu,
                                     scale=a2c2[:, h, 0:1],
                                     bias=a2c2[:, h, 1:2])
                (adde or nc.gpsimd).tensor_add(
                    og[:, h, :].rearrange("p (a b) -> p a b", a=2),
                    t[:], x2[:, h, ch * CS:(ch + 1) * CS]
                    .rearrange("p (a b) -> p a b", a=2))
                if ch == NCH - 1:
                    dstap = bass.AP(tensor=out_d[0].tensor,
                                    offset=h * S + ch * CS,
                                    ap=[[2 * S, C_], [1, CS]])
                    nc.sync.dma_start(out=dstap, in_=og[:, h, :])
                elif h == 1:
                    dstap = bass.AP(tensor=out_d[0].tensor,
                                    offset=ch * CS,
                                    ap=[[2 * S, C_], [S, 2], [1, CS]])
                    nc.sync.dma_start(out=dstap, in_=og[:])

            # ---- main loop, one chunk per cycle; the out conv for chunk
            #      ch-2 runs in chunk ch's tail so its q2 operands are long
            #      since finalized ----
            rcp2 = None
            for ch in range(NCH):
                b, cb = divmod(ch, 2)
                if True:
                    if cb == 0:
                        rcp2 = rcpp.tile([C_, 2, 2, CSB], dt.bfloat16,
                                         tag="rcp2", name="rcp2")
                    pd = None
                    for g in range(NG):
                        c0, ncg, P = _grp(g)
                        pz = psz.tile([C_, 2, 512], dt.float32, tag="pz",
                                      name="pz")
                        for k in range(2):
                            rows = slice(cb * CH_ROWS + k * ROWS_BK,
                                         cb * CH_ROWS + (k + 1) * ROWS_BK)
                            nc.tensor.matmul(
                                out=pz[0:P, k, 0:CSB],
                                lhsT=zb[0:P, g, 0:P],
                                rhs=st[g][:, rows, 0:W],
                                start=True, stop=True)
                        nc.scalar.activation(out=eq[g][:, cb, :, :],
                                             in_=pz[0:P, :, 0:CSB],
                                             func=AF.Exp,
                                             bias=cg9[0:P, g:g + 1])
                        if g == 0:
                            pd = psd.tile([C_, 2, 512], dt.float32,
                                          tag="pd", name="pd")
                        for k in range(2):
                            nc.tensor.matmul(
                                out=pd[:, k, 0:CSB],
                                lhsT=dones[0:P, g, :],
                                rhs=eq[g][:, cb, k, :],
                                start=(g == 0), stop=(g == NG - 1),
                                skip_group_check=True)
                        if ch >= 2:
                            if g <= 3:
                                out_mms(ch - 2, 0, g * 5, g * 5 + 5)
                            elif g == 4:
                                out_epi(ch - 2, 0)
                            elif g <= 8:
                                out_mms(ch - 2, 1, (g - 5) * 5, (g - 5) * 5 + 5)
                            else:
                                out_epi(ch - 2, 1)
                    with nc.allow_low_precision("softmax denom bf16"):
                        nc.vector.reciprocal(rcp2[:, cb, :, :],
                                             pd[:, :, 0:CSB])
                if ch >= NCH:
                    continue

                gengs = [nc.sync, nc.gpsimd]
                rcpfl = rcp2[:].rearrange("p a b c -> p (a b c)")
                reps = []
                for g in range(NG):
                    c0, ncg, P = _grp(g)
                    rep = repp.tile([126, 2, CSB], dt.bfloat16, tag="rep",
                                    name="rep")
                    reps.append(rep)
                    repsrc = bass.AP(
                        tensor=rcpfl.tensor,
                        offset=rcpfl.offset + c0 * (4 * CSB) + cb * (2 * CSB),
                        ap=[[4 * CSB, ncg], [0, 9], [1, 2 * CSB]])
                    gengs[(g + 1) % 2].dma_start(out=rep[0:P, :, :]
                                                 .rearrange("p a b -> p (a b)"),
                                                 in_=repsrc)
                for g in range(NG):
                    c0, ncg, P = _grp(g)
                    # q1 into the q2 tile (leaves eq free for the next exp)
                    nc.vector.tensor_mul(
                        q2[g][:, cb, :, :], eq[g][:, cb, :, :],
                        st[g][:, cb * CH_ROWS:(cb + 1) * CH_ROWS, 0:W])
                    # q2 in place
                    nc.vector.tensor_mul(
                        q2[g][:, cb, :, :], q2[g][:, cb, :, :],
                        reps[g][0:P, :, :])
                    if cb == 1 and b + 1 < NBLK:
                        strip_gather(b + 1, g, gengs[g % 2])

            for i, (ch, h) in enumerate([(NCH - 2, 0), (NCH - 2, 1),
                                         (NCH - 1, 0), (NCH - 1, 1)]):
                out_mms(ch, h, 0, 20, pool=psz if i % 2 else pso,
                        tag="pz" if i % 2 else "po")
                out_epi(ch, h, adde=nc.vector)

    _split_excess_waits(nc)
    return nc


def _split_excess_waits(nc):
    """This walrus build rejects >1 sync-wait per instruction; redistribute
    onto same-engine wait-nops inserted before."""
    import concourse.mybir as mybir
    cnt = [0]
    for bb in nc.main_func.blocks:
        new_list = []
        changed = False
        for ins in bb.instructions:
            si = ins.sync_info
            lim = 1
            if si is not None and si.on_wait is not None and len(si.on_wait) > lim:
                waits = list(si.on_wait)
                head, tail = waits[:-lim], waits[-lim:]
                for w in head:
                    nop = mybir.InstNoOp(name=f"waitsplit-{cnt[0]}", ins=[], outs=[])
                    cnt[0] += 1
                    nop.engine = ins.engine
                    nop.sync_info = mybir.SyncInfo(on_wait=[w], on_update=[])
                    nop.bass_nofuse = True
                    try:
                        nc.register_instruction(nop)
                    except Exception:
                        pass
                    new_list.append(nop)
                ins.sync_info = mybir.SyncInfo(
                    on_wait=tail, on_update=list(si.on_update or []))
                changed = True
            new_list.append(ins)
        if changed:
            bb.instructions[:] = new_list


def _get_program():
    global _PROGRAM
    if _PROGRAM is None:
        _PROGRAM = _build_program()
    return _PROGRAM


def _pack_inputs(x_b):
    """x_b: [C1, H*W] fp32 -> xs [128, 2, 6400] bf16."""
    bf16 = ml_dtypes.bfloat16
    xr = x_b.reshape(2, C_, S)            # [t, c, s]
    return np.ascontiguousarray(xr.transpose(1, 0, 2)).astype(bf16)


_IN_NAMES = ('w1t', 'c1sig', 'zb', 'dones', 'cg9', 'wc9', 'a2c2')


def kernel(**inputs):
    from concourse.bass_utils import run_bass_kernel_spmd

    x = np.asarray(inputs['x'], dtype=np.float32)
    cst = _fold_constants(**{k: np.asarray(v, dtype=np.float32)
                             for k, v in inputs.items() if k != 'x'})
    nc = _get_program()
    base = {k: cst[k] for k in _IN_NAMES}
    in_maps = [dict(base, xs=_pack_inputs(x[b].reshape(C1, H * W)))
               for b in range(B)]
    res = run_bass_kernel_spmd(nc, in_maps, list(range(B)))
    out = np.empty((B, C2, H, W), dtype=np.float32)
    for b in range(B):
        ob = res.results[b]['out'].astype(np.float32)     # [128, 2, 6400]
        out[b] = ob.transpose(1, 0, 2).reshape(C2, H, W)
    return out
